# revision 30
# baseline (speedup 1.0000x reference)
"""Multi-head attention (B=2, T=2048, D=1024, H=16) on 8 NeuronCores.

Sharding: core c handles batch b=c//4 and head-group g=c%4 (4 heads = 256
of the 1024 e-dims). QKV weights are column-sharded, w_o row-sharded.
The host transposes x and the weight shards (cast to bf16) so every device
matmul has its contraction dim on partitions with no on-device transposes.
Each core returns a [T, D] partial of the output projection; the host sums
the 4 partials per batch (the TP all-reduce) and folds in b_v @ w_o^T + b_o.

Device algorithm (per core):
  Minimal prologue: only K-chunk0/Q-chunk0 projections run before the
  attention stream starts (~10us); the remaining K/Q projection chunks are
  DRIPPED into the s-loops through a dedicated 1-bank psum slot ("kq"),
  paced so each chunk lands just before the scores that need it. x stays
  resident in SBUF (bf16) and the V projection streams from it inside the
  first half-block's s-loop.
  Each 512-wide t-block is split into two HEAD-PAIR halves (half m covers
  heads 2m, 2m+1 -> outT[:, m, :]). Per half, per 128-wide s-tile:
    scores^T psum [s, 2, t] via 2 matmuls (head hh at KT/QT partition base
    64*hh), prefetched one iteration ahead (incl. across half boundaries)
    so the exp stream never waits on PE; ONE Exp activation (free-1024) ->
    pT bf16; P@V via per-head [128, 65] lhsT = [V_h | ones]: psum row 64
    accumulates the softmax denominator for free (NO separate denominator
    matmuls), trailing exp by two s-tiles.
  Tail per half: both pv banks staged to SBUF immediately (frees the
  2-deep pv ring for the next half), 1/den -> DRAM-bounce broadcast,
  head-odd DMA-shifted (sbuf->sbuf) to lanes 64:128, normalize into outT.
  Output projection: per [128 t, 512 f] block, ONE accumulation group of
  2 full-128-contraction matmuls (head pair fused via outT/wo_sb layout),
  dripped into the NEXT t-block's ACT-bound s-loops.
  PSUM: scores 2x[128,2,512] (4) + pv ring 2 + kq 1 + y/V 1 = 8 banks.
"""

import sys
from contextlib import ExitStack

import numpy as np

try:
    import concourse.bass as bass
except ImportError:  # pragma: no cover
    sys.path.insert(0, "/opt/trn_rl_repo")
    import concourse.bass as bass

import concourse.tile as tile
from concourse import mybir
from concourse.bass_utils import run_bass_kernel_spmd

F32 = mybir.dt.float32
F32R = mybir.dt.float32r
BF16 = mybir.dt.bfloat16

D = 1024
H = 16
DK = 64
E = 256  # per-core out-dim of the head group (4 heads x 64)
P = 128
N_CORES = 8


def _split_multi_waits(nc):
    """This container's walrus encodes at most ONE sync-wait per instruction
    ("Too many sync wait commands" in codegen otherwise). Tile attaches
    multi-sem waits to instructions; hoist all but the last wait onto
    standalone single-wait EventSemaphore instructions inserted just before,
    on the same engine — semantically identical (engine stalls in order)."""
    n = 0
    for fn in nc.m.functions:
        for bb in fn.blocks:
            il = bb.instructions
            i = 0
            while i < len(il):
                ins = il[i]
                si = ins.sync_info
                if si is not None and si.on_wait and len(si.on_wait) > 1:
                    waits = list(si.on_wait)
                    for k, w in enumerate(waits[:-1]):
                        ev = mybir.InstEventSemaphore(
                            name=f"{ins.name}_w{k}", ins=[], outs=[],
                            sync_info=mybir.SyncInfo(on_wait=[w], on_update=[]),
                        )
                        ev.engine = ins.engine
                        nc.register_instruction(ev)
                        il.insert(i, ev)
                        i += 1
                        n += 1
                    si.on_wait = waits[-1:]
                i += 1
    return n


def build_nc(T=2048, TB=512):
    """Build the SPMD Bass program (identical on all 8 cores)."""
    NT = T // P       # number of 128-wide s-tiles / t-tiles
    NTB = T // TB     # number of t-blocks in phase 2
    NPB = T // 512    # number of 512-wide t-chunks for projections

    nc = bass.Bass()

    xT_d = nc.dram_tensor("xT", [D, T], BF16, kind="ExternalInput")
    wqT_d = nc.dram_tensor("wqT", [D, E], BF16, kind="ExternalInput")
    wkT_d = nc.dram_tensor("wkT", [D, E], BF16, kind="ExternalInput")
    wvT_d = nc.dram_tensor("wvT", [D, E], BF16, kind="ExternalInput")
    wo_d = nc.dram_tensor("wo_sh", [E, D], F32R, kind="ExternalInput")
    bq_d = nc.dram_tensor("bq2", [P, 2], F32, kind="ExternalInput")
    bk_d = nc.dram_tensor("bk2", [P, 2], F32, kind="ExternalInput")
    y_d = nc.dram_tensor("y", [T, D], BF16, kind="ExternalOutput")
    den_dram = nc.dram_tensor("den_scratch", [NTB * 2, 2, TB], F32)

    with tile.TileContext(nc) as tc:
        with tc.tile_pool(name="const", bufs=1) as const:
            QT = const.tile([P, 2, T], F32R)       # [e%128, pair, t]
            KT = const.tile([P, 2, T], F32R)
            V = const.tile([P, NT, 4, DK + 1], BF16)  # [s%128, s//128, h, dk|1]
            pT = const.tile([P, NT, 2, TB], BF16)  # exp(scores^T) of one half
            outT = const.tile([P, 2, T], F32R)     # normalized (attn @ V)^T
            wo_sb = const.tile([P, 2, D], F32R)
            bq_sb = const.tile([P, 2], F32)
            bk_sb = const.tile([P, 2], F32)

            # ones column: pv psum row 64 accumulates the softmax denominator
            nc.vector.memset(V[:, :, :, DK:DK + 1], 1.0)

            # x + projection weights stay resident in SBUF (bf16, ~44KB)
            xT_sb = const.tile([P, 8, T], BF16)
            wv_sb = const.tile([P, 8, E], BF16)
            wk_sb = const.tile([P, 8, E], BF16)
            wq_sb = const.tile([P, 8, E], BF16)

            # DMA order = need order: wk+x0 gate the prologue, wq gates
            # Q0, wv the in-loop V projection, x1-3 dripped K chunks
            nc.sync.dma_start(out=wk_sb, in_=wkT_d[:].rearrange("(dt p) e -> p dt e", p=P))
            for dt in range(8):
                nc.sync.dma_start(
                    out=xT_sb[:, dt, 0:512], in_=xT_d[dt * P:(dt + 1) * P, 0:512]
                )
            nc.sync.dma_start(out=wq_sb, in_=wqT_d[:].rearrange("(dt p) e -> p dt e", p=P))
            nc.sync.dma_start(out=wv_sb, in_=wvT_d[:].rearrange("(dt p) e -> p dt e", p=P))
            nc.sync.dma_start(out=bq_sb, in_=bq_d[:])
            nc.sync.dma_start(out=bk_sb, in_=bk_d[:])
            for t4 in range(1, NPB):
                for dt in range(8):
                    nc.sync.dma_start(
                        out=xT_sb[:, dt, t4 * 512:(t4 + 1) * 512],
                        in_=xT_d[dt * P:(dt + 1) * P, t4 * 512:(t4 + 1) * 512],
                    )
            # wo is not needed until the first output projection
            nc.sync.dma_start(out=wo_sb, in_=wo_d[:].rearrange("(m p) f -> p m f", p=P))

            # ---- prologue: K chunk0 + Q chunk0 only (via a scoped pool) ----
            with tc.tile_pool(name="ps1", bufs=4, space="PSUM") as ps1:
                # p-state warmup: the PE runs at half rate until it has been
                # continuously busy for 3us. Dummy matmuls during the initial
                # DMA-idle window ramp it up so the real prologue projections
                # run at full rate (outputs discarded).
                warm = const.tile([P, 512], BF16)
                nc.vector.memset(warm, 0.0)
                wps = ps1.tile([P, 512], F32, tag="warm", name="warm_ps")
                for i in range(10):
                    nc.tensor.matmul(
                        wps[0:16, :],
                        lhsT=warm[:, 0:16],
                        rhs=warm[:, :],
                        start=True,
                        stop=True,
                        skip_group_check=True,
                    )
                for w_sb, dst, b_sb, t4, em in (
                    (wk_sb, KT, bk_sb, 0, 0),
                    (wq_sb, QT, bq_sb, 0, 0),
                    (wq_sb, QT, bq_sb, 0, 1),
                    (wk_sb, KT, bk_sb, 0, 1),
                ):
                    ps = ps1.tile([P, 512], F32, tag="proj", name="proj_ps")
                    for dt in range(8):
                        nc.tensor.matmul(
                            ps,
                            lhsT=w_sb[:, dt, em * P:(em + 1) * P],
                            rhs=xT_sb[:, dt, t4 * 512:(t4 + 1) * 512],
                            start=(dt == 0),
                            stop=(dt == 7),
                        )
                    nc.vector.tensor_scalar_add(
                        out=dst[:, em, t4 * 512:(t4 + 1) * 512],
                        in0=ps,
                        scalar1=b_sb[:, em:em + 1],
                    )

            # -------- phase 2: attention + fused output projection --------
            with (
                tc.tile_pool(name="p2", bufs=1) as p2,
                tc.tile_pool(name="ps_sc", bufs=2, space="PSUM") as ps_sc,
                tc.tile_pool(name="ps_pv", bufs=2, space="PSUM") as ps_pv,
                tc.tile_pool(name="ps_y", bufs=1, space="PSUM") as ps_y,
            ):
                # ---- dripped projection chunks (the "kq" 1-bank slot) ----
                # each group = one [128e, 512t] projection accumulation,
                # emitted in slices of `dts` so it hides in s-loop slack
                class Group:
                    def __init__(self, w_sb, dst, b_sb, t4, em):
                        self.w_sb, self.dst, self.b_sb = w_sb, dst, b_sb
                        self.t4, self.em = t4, em
                        self.ps = None

                    def emit(self, dts, last):
                        if self.ps is None:
                            self.ps = ps_pv.tile([P, 512], F32, tag="kq",
                                                 bufs=1, name="kq_ps")
                        for dt in dts:
                            nc.tensor.matmul(
                                self.ps,
                                lhsT=self.w_sb[:, dt, self.em * P:(self.em + 1) * P],
                                rhs=xT_sb[:, dt, self.t4 * 512:(self.t4 + 1) * 512],
                                start=(dt == 0),
                                stop=(dt == 7),
                            )
                        if last:
                            nc.vector.tensor_scalar_add(
                                out=self.dst[:, self.em,
                                             self.t4 * 512:(self.t4 + 1) * 512],
                                in0=self.ps,
                                scalar1=self.b_sb[:, self.em:self.em + 1],
                            )

                def KG(t4, em):
                    return Group(wk_sb, KT, bk_sb, t4, em)

                def QG(t4, em):
                    return Group(wq_sb, QT, bq_sb, t4, em)

                # drip_plan[hi][st] = (group, dts, last)
                drip_plan = {}

                def plan(hi, g, st0):
                    drip_plan.setdefault(hi, {})[st0] = (g, range(0, 4), False)
                    drip_plan[hi][st0 + 1] = (g, range(4, 8), True)

                # half 0 absorbs K chunks 1-3 (em0 just ahead of its own
                # scores, em1 before half 1 needs them)
                plan(0, KG(1, 0), 1)
                plan(0, KG(1, 1), 3)
                plan(0, KG(2, 0), 5)
                plan(0, KG(2, 1), 7)
                plan(0, KG(3, 0), 9)
                plan(0, KG(3, 1), 11)
                # Q chunk c lands in the halves of t-block c-1
                plan(1, QG(1, 0), 2)
                plan(1, QG(1, 1), 6)
                plan(2, QG(2, 0), 2)
                plan(3, QG(2, 1), 2)
                plan(4, QG(3, 0), 2)
                plan(5, QG(3, 1), 2)

                def y_unit(tt, fb, mk=None, act_copy=False):
                    # output projection for one [128 t, 512 f] block: one
                    # accumulation group of 2 full-128-contraction matmuls
                    # (head pair via outT/wo_sb layout), copy to SBUF, DMA
                    f0 = fb * 512
                    if mk is None:
                        yps = ps_y.tile([P, 512], F32, tag="y", name="y_ps")
                    else:
                        yps = mk()
                    for m2 in range(2):
                        nc.tensor.matmul(
                            yps,
                            lhsT=outT[:, m2, tt * P:(tt + 1) * P],
                            rhs=wo_sb[:, m2, f0:f0 + 512],
                            start=(m2 == 0),
                            stop=(m2 == 1),
                            skip_group_check=True,
                        )
                    ysb = p2.tile([P, 512], BF16, tag="ysb", bufs=3, name="ysb")
                    if act_copy:
                        nc.scalar.copy(out=ysb, in_=yps)
                    else:
                        nc.vector.tensor_copy(out=ysb, in_=yps)
                    nc.sync.dma_start(
                        out=y_d[tt * P:(tt + 1) * P, f0:f0 + 512], in_=ysb
                    )

                def emit_scores(m, t0, st):
                    sc = ps_sc.tile([P, 2, TB], F32, tag="sc", name="sc_ps")
                    for hh in range(2):
                        p0 = DK * hh
                        nc.tensor.matmul(
                            sc[:, hh, :],
                            lhsT=KT[p0:p0 + DK, m, st * P:(st + 1) * P],
                            rhs=QT[p0:p0 + DK, m, t0:t0 + TB],
                            start=True,
                            stop=True,
                        )
                    return sc

                halves = [(tb, m) for tb in range(NTB) for m in range(2)]
                pending = []  # deferred y-units of the previous t-block
                sc_cur = emit_scores(halves[0][1], halves[0][0] * TB, 0)
                for hi, (tb, m) in enumerate(halves):
                    t0 = tb * TB
                    pvA = ps_pv.tile([P, TB], F32, tag="pv", name="pvA")
                    pvB = ps_pv.tile([P, TB], F32, tag="pv", name="pvB")

                    def pv_dn(st):
                        for hh, pv in ((0, pvA), (1, pvB)):
                            nc.tensor.matmul(
                                pv[0:DK + 1, :],
                                lhsT=V[:, st, 2 * m + hh, :],
                                rhs=pT[:, st, hh, :],
                                start=(st == 0),
                                stop=(st == NT - 1),
                                skip_group_check=True,
                            )

                    for st in range(NT):
                        # scores are emitted one iteration AHEAD (incl.
                        # across half boundaries) so the exp stream never
                        # waits on PE
                        if st + 1 < NT:
                            sc_nxt = emit_scores(m, t0, st + 1)
                        elif hi + 1 < len(halves):
                            tb2, m2 = halves[hi + 1]
                            sc_nxt = emit_scores(m2, tb2 * TB, 0)
                        else:
                            sc_nxt = None
                        if hi == 0:
                            # V projection from the resident bf16 x, one
                            # s-chunk per iteration, psum via the y bank
                            vps = ps_y.tile([P, 512], F32, tag="y", name="v_ps")
                            for dt in range(8):
                                nc.tensor.matmul(
                                    vps[:, :E],
                                    lhsT=xT_sb[:, dt, st * P:(st + 1) * P],
                                    rhs=wv_sb[:, dt, :],
                                    start=(dt == 0),
                                    stop=(dt == 7),
                                )
                            nc.vector.tensor_copy(
                                out=V[:, st, :, 0:DK], in_=vps[:, :E]
                            )
                        # software pipeline (depth 2): P@V trails exp by two
                        # s-tiles so a new half's first pv matmul never waits
                        # on the previous half's pv banks still draining
                        if st > 1:
                            pv_dn(st - 2)
                        # dripped projection slice for this iteration
                        if hi in drip_plan and st in drip_plan[hi]:
                            g, dts, last = drip_plan[hi][st]
                            g.emit(dts, last)
                        nc.scalar.activation(
                            out=pT[:, st, :, :],
                            in_=sc_cur,
                            func=mybir.ActivationFunctionType.Exp,
                            scale=0.125,
                        )
                        sc_cur = sc_nxt
                        # drip the previous t-block's output projection into
                        # this (ACT-bound) s-loop
                        if pending and st in (5, 8, 11, 14):
                            y_unit(*pending.pop(0))
                    pv_dn(NT - 2)
                    pv_dn(NT - 1)
                    # tail: stage both pv banks to SBUF immediately (frees
                    # the 2-deep psum ring for the next half), 1/den rows ->
                    # DRAM-bounce broadcast, normalize from the SBUF copies;
                    # head-odd is DMA-shifted (sbuf->sbuf) to lanes 64:128
                    ouA = p2.tile([P, TB], F32, tag="ouA", bufs=2, name="ouA")
                    ouB = p2.tile([P, TB], F32R, tag="ouB", bufs=2, name="ouB")
                    den_inv = p2.tile([P, 2, TB], F32, tag="den_inv",
                                      bufs=2, name="den_inv")
                    last = hi == len(halves) - 1
                    if last:
                        # no next half to feed: reciprocals straight from
                        # PSUM, ahead of the staging copies, shorten the
                        # tail's den -> rep -> normalize chain
                        for hh, pv in ((0, pvA), (1, pvB)):
                            nc.vector.reciprocal(
                                out=den_inv[DK:DK + 1, hh, :],
                                in_=pv[DK:DK + 1, :],
                            )
                            nc.sync.dma_start(
                                out=den_dram[2 * tb + m, hh:hh + 1, :],
                                in_=den_inv[DK:DK + 1, hh, :],
                            )
                    nc.vector.tensor_copy(out=ouA[0:DK + 1, :], in_=pvA[0:DK + 1, :])
                    nc.vector.tensor_copy(out=ouB[0:DK + 1, :], in_=pvB[0:DK + 1, :])
                    if not last:
                        # mid-halves read the SBUF staging copy so the pv
                        # bank is released by the copy alone (2-deep ring)
                        for hh, ou in ((0, ouA), (1, ouB)):
                            nc.vector.reciprocal(
                                out=den_inv[DK:DK + 1, hh, :],
                                in_=ou[DK:DK + 1, :],
                            )
                            nc.sync.dma_start(
                                out=den_dram[2 * tb + m, hh:hh + 1, :],
                                in_=den_inv[DK:DK + 1, hh, :],
                            )
                    rep = p2.tile([P, TB], F32, tag="rep", bufs=2, name="rep")
                    for hh in range(2):
                        nc.sync.dma_start(
                            out=rep[DK * hh:DK * hh + DK, :],
                            in_=den_dram[2 * tb + m, hh:hh + 1, :].to_broadcast([DK, TB]),
                        )
                    ou2 = p2.tile([P, TB], F32R, tag="ou2", bufs=2, name="ou2")
                    nc.sync.dma_start(out=ou2[DK:P, :], in_=ouB[0:DK, :])
                    nc.vector.tensor_mul(
                        outT[0:DK, m, t0:t0 + TB], ouA[0:DK, :], rep[0:DK, :]
                    )
                    nc.vector.tensor_mul(
                        outT[DK:P, m, t0:t0 + TB], ou2[DK:P, :], rep[DK:P, :]
                    )
                    if m == 1:
                        pending = [(tt, fb)
                                   for tt in range(tb * (TB // P), (tb + 1) * (TB // P))
                                   for fb in range(2)]
                # tail units: the pv ring, kq and score banks are free now --
                # spread across 6 banks so the units pipeline instead of
                # serializing; ACT is idle after the final exp, so it does
                # the PSUM->SBUF copies
                banks = [
                    lambda: ps_y.tile([P, 512], F32, tag="y", name="y_ps"),
                    lambda: ps_pv.tile([P, TB], F32, tag="pv", name="y_ps"),
                    lambda: ps_pv.tile([P, TB], F32, tag="pv", name="y_ps"),
                    lambda: ps_pv.tile([P, 512], F32, tag="kq", bufs=1, name="y_ps"),
                    lambda: ps_sc.tile([P, 2, TB], F32, tag="sc", name="y_ps")[:, 0, :],
                    lambda: ps_sc.tile([P, 2, TB], F32, tag="sc", name="y_ps")[:, 0, :],
                ]
                for i, u in enumerate(pending):
                    y_unit(*u, mk=banks[i % 6], act_copy=(i % 2 == 0))
    _split_multi_waits(nc)
    return nc


def _shard_inputs(x, w_q, b_q, w_k, b_k, w_v, b_v, w_o, b_o):
    import ml_dtypes
    bf16 = ml_dtypes.bfloat16
    in_maps = []
    for c in range(N_CORES):
        b, g = c // 4, c % 4
        sl = slice(g * E, (g + 1) * E)
        in_maps.append({
            "xT": np.ascontiguousarray(x[b].T).astype(bf16),
            "wqT": np.ascontiguousarray(w_q[sl, :].T).astype(bf16),
            "wkT": np.ascontiguousarray(w_k[sl, :].T).astype(bf16),
            "wvT": np.ascontiguousarray(w_v[sl, :].T).astype(bf16),
            "wo_sh": np.ascontiguousarray(w_o[:, sl].T, dtype=np.float32),
            "bq2": np.ascontiguousarray(b_q[sl].reshape(2, P).T, dtype=np.float32),
            "bk2": np.ascontiguousarray(b_k[sl].reshape(2, P).T, dtype=np.float32),
        })
    return in_maps


_NC_CACHE = {}


def kernel(x, w_q, b_q, w_k, b_k, w_v, b_v, w_o, b_o, _trace=False):
    x = np.asarray(x, dtype=np.float32)
    B, T, _ = x.shape
    args = [np.asarray(a, dtype=np.float32)
            for a in (w_q, b_q, w_k, b_k, w_v, b_v, w_o, b_o)]
    w_q, b_q, w_k, b_k, w_v, b_v, w_o, b_o = args

    if T not in _NC_CACHE:
        _NC_CACHE[T] = build_nc(T=T)
    nc = _NC_CACHE[T]
    in_maps = _shard_inputs(x, w_q, b_q, w_k, b_k, w_v, b_v, w_o, b_o)
    res = run_bass_kernel_spmd(nc, in_maps, list(range(N_CORES)), trace=_trace)

    y = np.zeros((B, T, D), dtype=np.float32)
    for c in range(N_CORES):
        y[c // 4] += np.asarray(res.results[c]["y"], dtype=np.float32)
    fold = b_v @ w_o.T + b_o
    y += fold[None, None, :]
    if _trace:
        return y, res
    return y


# revision 39
# speedup vs baseline: 1.0044x; 1.0044x over previous
"""Multi-head attention (B=2, T=2048, D=1024, H=16) on 8 NeuronCores.

Sharding: core c handles batch b=c//4 and head-group g=c%4 (4 heads = 256
of the 1024 e-dims). QKV weights are column-sharded, w_o row-sharded.
The host transposes x and the weight shards (cast to bf16) so every device
matmul has its contraction dim on partitions with no on-device transposes.
Each core returns a [T, D] partial of the output projection; the host sums
the 4 partials per batch (the TP all-reduce) and folds in b_v @ w_o^T + b_o.

Device algorithm (per core):
  Minimal prologue: only K-chunk0/Q-chunk0 projections run before the
  attention stream starts (~10us); the remaining K/Q projection chunks are
  DRIPPED into the s-loops through a dedicated 1-bank psum slot ("kq"),
  paced so each chunk lands just before the scores that need it. x stays
  resident in SBUF (bf16) and the V projection streams from it inside the
  first half-block's s-loop.
  Each 512-wide t-block is split into two HEAD-PAIR halves (half m covers
  heads 2m, 2m+1 -> outT[:, m, :]). Per half, per 128-wide s-tile:
    scores^T psum [s, 2, t] via 2 matmuls (head hh at KT/QT partition base
    64*hh), prefetched one iteration ahead (incl. across half boundaries)
    so the exp stream never waits on PE; ONE Exp activation (free-1024) ->
    pT bf16; P@V via per-head [128, 65] lhsT = [V_h | ones]: psum row 64
    accumulates the softmax denominator for free (NO separate denominator
    matmuls), trailing exp by two s-tiles.
  Tail per half: both pv banks staged to SBUF immediately (frees the
  2-deep pv ring for the next half), 1/den -> DRAM-bounce broadcast,
  head-odd DMA-shifted (sbuf->sbuf) to lanes 64:128, normalize into outT.
  Output projection: per [128 t, 512 f] block, ONE accumulation group of
  2 full-128-contraction matmuls (head pair fused via outT/wo_sb layout),
  dripped into the NEXT t-block's ACT-bound s-loops.
  PSUM: scores 2x[128,2,512] (4) + pv ring 2 + kq 1 + y/V 1 = 8 banks.
"""

import sys
from contextlib import ExitStack

import numpy as np

try:
    import concourse.bass as bass
except ImportError:  # pragma: no cover
    sys.path.insert(0, "/opt/trn_rl_repo")
    import concourse.bass as bass

import concourse.tile as tile
from concourse import mybir
from concourse.bass_utils import run_bass_kernel_spmd

F32 = mybir.dt.float32
F32R = mybir.dt.float32r
BF16 = mybir.dt.bfloat16

D = 1024
H = 16
DK = 64
E = 256  # per-core out-dim of the head group (4 heads x 64)
P = 128
N_CORES = 8


def _split_multi_waits(nc):
    """This container's walrus encodes at most ONE sync-wait per instruction
    ("Too many sync wait commands" in codegen otherwise). Tile attaches
    multi-sem waits to instructions; hoist all but the last wait onto
    standalone single-wait EventSemaphore instructions inserted just before,
    on the same engine — semantically identical (engine stalls in order)."""
    n = 0
    for fn in nc.m.functions:
        for bb in fn.blocks:
            il = bb.instructions
            i = 0
            while i < len(il):
                ins = il[i]
                si = ins.sync_info
                if si is not None and si.on_wait and len(si.on_wait) > 1:
                    waits = list(si.on_wait)
                    for k, w in enumerate(waits[:-1]):
                        ev = mybir.InstEventSemaphore(
                            name=f"{ins.name}_w{k}", ins=[], outs=[],
                            sync_info=mybir.SyncInfo(on_wait=[w], on_update=[]),
                        )
                        ev.engine = ins.engine
                        nc.register_instruction(ev)
                        il.insert(i, ev)
                        i += 1
                        n += 1
                    si.on_wait = waits[-1:]
                i += 1
    return n


def build_nc(T=2048, TB=512):
    """Build the SPMD Bass program (identical on all 8 cores)."""
    NT = T // P       # number of 128-wide s-tiles / t-tiles
    NTB = T // TB     # number of t-blocks in phase 2
    NPB = T // 512    # number of 512-wide t-chunks for projections

    nc = bass.Bass()

    xT_d = nc.dram_tensor("xT", [D, T], BF16, kind="ExternalInput")
    wqT_d = nc.dram_tensor("wqT", [D, E], BF16, kind="ExternalInput")
    wkT_d = nc.dram_tensor("wkT", [D, E], BF16, kind="ExternalInput")
    wvT_d = nc.dram_tensor("wvT", [D, E], BF16, kind="ExternalInput")
    wo_d = nc.dram_tensor("wo_sh", [E, D], F32R, kind="ExternalInput")
    bq_d = nc.dram_tensor("bq2", [P, 2], F32, kind="ExternalInput")
    bk_d = nc.dram_tensor("bk2", [P, 2], F32, kind="ExternalInput")
    y_d = nc.dram_tensor("y", [T, D], BF16, kind="ExternalOutput")
    den_dram = nc.dram_tensor("den_scratch", [NTB * 2, 2, TB], F32)

    with tile.TileContext(nc) as tc:
        with tc.tile_pool(name="const", bufs=1) as const:
            QT = const.tile([P, 2, T], F32R)       # [e%128, pair, t]
            KT = const.tile([P, 2, T], F32R)
            V = const.tile([P, NT, 4, DK + 1], BF16)  # [s%128, s//128, h, dk|1]
            pT = const.tile([P, NT, 2, TB], BF16)  # exp(scores^T) of one half
            outT = const.tile([P, 2, T], F32R)     # normalized (attn @ V)^T
            wo_sb = const.tile([P, 2, D], F32R)
            bq_sb = const.tile([P, 2], F32)
            bk_sb = const.tile([P, 2], F32)

            # ones column: pv psum row 64 accumulates the softmax denominator
            nc.vector.memset(V[:, :, :, DK:DK + 1], 1.0)

            # x + projection weights stay resident in SBUF (bf16, ~44KB)
            xT_sb = const.tile([P, 8, T], BF16)
            wv_sb = const.tile([P, 8, E], BF16)
            wk_sb = const.tile([P, 8, E], BF16)
            wq_sb = const.tile([P, 8, E], BF16)

            # DMA order = need order: wk+x0 gate the prologue, wq gates
            # Q0, wv the in-loop V projection, x1-3 dripped K chunks
            nc.sync.dma_start(out=wk_sb, in_=wkT_d[:].rearrange("(dt p) e -> p dt e", p=P))
            for dt in range(8):
                nc.sync.dma_start(
                    out=xT_sb[:, dt, 0:512], in_=xT_d[dt * P:(dt + 1) * P, 0:512]
                )
            nc.sync.dma_start(out=wq_sb, in_=wqT_d[:].rearrange("(dt p) e -> p dt e", p=P))
            nc.sync.dma_start(out=wv_sb, in_=wvT_d[:].rearrange("(dt p) e -> p dt e", p=P))
            nc.sync.dma_start(out=bq_sb, in_=bq_d[:])
            nc.sync.dma_start(out=bk_sb, in_=bk_d[:])
            for t4 in range(1, NPB):
                for dt in range(8):
                    nc.sync.dma_start(
                        out=xT_sb[:, dt, t4 * 512:(t4 + 1) * 512],
                        in_=xT_d[dt * P:(dt + 1) * P, t4 * 512:(t4 + 1) * 512],
                    )
            # wo is not needed until the first output projection
            nc.sync.dma_start(out=wo_sb, in_=wo_d[:].rearrange("(m p) f -> p m f", p=P))

            # ---- prologue: K chunk0 + Q chunk0 only (via a scoped pool) ----
            with tc.tile_pool(name="ps1", bufs=4, space="PSUM") as ps1:
                for w_sb, dst, b_sb, t4, em in (
                    (wk_sb, KT, bk_sb, 0, 0),
                    (wq_sb, QT, bq_sb, 0, 0),
                    (wq_sb, QT, bq_sb, 0, 1),
                    (wk_sb, KT, bk_sb, 0, 1),
                ):
                    ps = ps1.tile([P, 512], F32, tag="proj", name="proj_ps")
                    for dt in range(8):
                        nc.tensor.matmul(
                            ps,
                            lhsT=w_sb[:, dt, em * P:(em + 1) * P],
                            rhs=xT_sb[:, dt, t4 * 512:(t4 + 1) * 512],
                            start=(dt == 0),
                            stop=(dt == 7),
                        )
                    nc.vector.tensor_scalar_add(
                        out=dst[:, em, t4 * 512:(t4 + 1) * 512],
                        in0=ps,
                        scalar1=b_sb[:, em:em + 1],
                    )

            # -------- phase 2: attention + fused output projection --------
            with (
                tc.tile_pool(name="p2", bufs=1) as p2,
                tc.tile_pool(name="ps_sc", bufs=2, space="PSUM") as ps_sc,
                tc.tile_pool(name="ps_pv", bufs=2, space="PSUM") as ps_pv,
                tc.tile_pool(name="ps_y", bufs=1, space="PSUM") as ps_y,
            ):
                # ---- dripped projection chunks (the "kq" 1-bank slot) ----
                # each group = one [128e, 512t] projection accumulation,
                # emitted in slices of `dts` so it hides in s-loop slack
                class Group:
                    def __init__(self, w_sb, dst, b_sb, t4, em):
                        self.w_sb, self.dst, self.b_sb = w_sb, dst, b_sb
                        self.t4, self.em = t4, em
                        self.ps = None

                    def emit(self, dts, last):
                        if self.ps is None:
                            self.ps = ps_pv.tile([P, 512], F32, tag="kq",
                                                 bufs=1, name="kq_ps")
                        for dt in dts:
                            nc.tensor.matmul(
                                self.ps,
                                lhsT=self.w_sb[:, dt, self.em * P:(self.em + 1) * P],
                                rhs=xT_sb[:, dt, self.t4 * 512:(self.t4 + 1) * 512],
                                start=(dt == 0),
                                stop=(dt == 7),
                            )
                        if last:
                            nc.vector.tensor_scalar_add(
                                out=self.dst[:, self.em,
                                             self.t4 * 512:(self.t4 + 1) * 512],
                                in0=self.ps,
                                scalar1=self.b_sb[:, self.em:self.em + 1],
                            )

                def KG(t4, em):
                    return Group(wk_sb, KT, bk_sb, t4, em)

                def QG(t4, em):
                    return Group(wq_sb, QT, bq_sb, t4, em)

                # drip_plan[hi][st] = (group, dts, last)
                drip_plan = {}

                def plan(hi, g, st0):
                    drip_plan.setdefault(hi, {})[st0] = (g, range(0, 4), False)
                    drip_plan[hi][st0 + 1] = (g, range(4, 8), True)

                # half 0 absorbs K chunks 1-3 (em0 just ahead of its own
                # scores, em1 before half 1 needs them)
                plan(0, KG(1, 0), 1)
                plan(0, KG(1, 1), 3)
                plan(0, KG(2, 0), 5)
                plan(0, KG(2, 1), 7)
                plan(0, KG(3, 0), 9)
                plan(0, KG(3, 1), 11)
                # Q chunk c lands in the halves of t-block c-1
                plan(1, QG(1, 0), 2)
                plan(1, QG(1, 1), 6)
                plan(2, QG(2, 0), 2)
                plan(3, QG(2, 1), 2)
                plan(4, QG(3, 0), 2)
                plan(5, QG(3, 1), 2)

                def y_unit(tt, fb, mk=None, act_copy=False):
                    # output projection for one [128 t, 512 f] block: one
                    # accumulation group of 2 full-128-contraction matmuls
                    # (head pair via outT/wo_sb layout), copy to SBUF, DMA
                    f0 = fb * 512
                    if mk is None:
                        yps = ps_y.tile([P, 512], F32, tag="y", name="y_ps")
                    else:
                        yps = mk()
                    for m2 in range(2):
                        nc.tensor.matmul(
                            yps,
                            lhsT=outT[:, m2, tt * P:(tt + 1) * P],
                            rhs=wo_sb[:, m2, f0:f0 + 512],
                            start=(m2 == 0),
                            stop=(m2 == 1),
                            skip_group_check=True,
                        )
                    ysb = p2.tile([P, 512], BF16, tag="ysb", bufs=3, name="ysb")
                    if act_copy:
                        nc.scalar.copy(out=ysb, in_=yps)
                    else:
                        nc.vector.tensor_copy(out=ysb, in_=yps)
                    nc.sync.dma_start(
                        out=y_d[tt * P:(tt + 1) * P, f0:f0 + 512], in_=ysb
                    )

                def emit_scores(m, t0, st):
                    sc = ps_sc.tile([P, 2, TB], F32, tag="sc", name="sc_ps")
                    for hh in range(2):
                        p0 = DK * hh
                        nc.tensor.matmul(
                            sc[:, hh, :],
                            lhsT=KT[p0:p0 + DK, m, st * P:(st + 1) * P],
                            rhs=QT[p0:p0 + DK, m, t0:t0 + TB],
                            start=True,
                            stop=True,
                        )
                    return sc

                halves = [(tb, m) for tb in range(NTB) for m in range(2)]
                pending = []  # deferred y-units of the previous t-block
                sc_cur = emit_scores(halves[0][1], halves[0][0] * TB, 0)
                for hi, (tb, m) in enumerate(halves):
                    t0 = tb * TB
                    pvA = ps_pv.tile([P, TB], F32, tag="pv", name="pvA")
                    pvB = ps_pv.tile([P, TB], F32, tag="pv", name="pvB")

                    def pv_dn(st):
                        for hh, pv in ((0, pvA), (1, pvB)):
                            nc.tensor.matmul(
                                pv[0:DK + 1, :],
                                lhsT=V[:, st, 2 * m + hh, :],
                                rhs=pT[:, st, hh, :],
                                start=(st == 0),
                                stop=(st == NT - 1),
                                skip_group_check=True,
                            )

                    for st in range(NT):
                        # scores are emitted one iteration AHEAD (incl.
                        # across half boundaries) so the exp stream never
                        # waits on PE
                        if st + 1 < NT:
                            sc_nxt = emit_scores(m, t0, st + 1)
                        elif hi + 1 < len(halves):
                            tb2, m2 = halves[hi + 1]
                            sc_nxt = emit_scores(m2, tb2 * TB, 0)
                        else:
                            sc_nxt = None
                        if hi == 0:
                            # V projection from the resident bf16 x, one
                            # s-chunk per iteration, psum via the y bank
                            vps = ps_y.tile([P, 512], F32, tag="y", name="v_ps")
                            for dt in range(8):
                                nc.tensor.matmul(
                                    vps[:, :E],
                                    lhsT=xT_sb[:, dt, st * P:(st + 1) * P],
                                    rhs=wv_sb[:, dt, :],
                                    start=(dt == 0),
                                    stop=(dt == 7),
                                )
                            nc.vector.tensor_copy(
                                out=V[:, st, :, 0:DK], in_=vps[:, :E]
                            )
                        # software pipeline (depth 2): P@V trails exp by two
                        # s-tiles so a new half's first pv matmul never waits
                        # on the previous half's pv banks still draining
                        if st > 1:
                            pv_dn(st - 2)
                        # dripped projection slice for this iteration
                        if hi in drip_plan and st in drip_plan[hi]:
                            g, dts, last = drip_plan[hi][st]
                            g.emit(dts, last)
                        nc.scalar.activation(
                            out=pT[:, st, :, :],
                            in_=sc_cur,
                            func=mybir.ActivationFunctionType.Exp,
                            scale=0.125,
                        )
                        sc_cur = sc_nxt
                        # drip the previous t-block's output projection into
                        # this (ACT-bound) s-loop
                        if pending and st in (5, 8, 11, 14):
                            y_unit(*pending.pop(0))
                    pv_dn(NT - 2)
                    pv_dn(NT - 1)
                    # tail: stage both pv banks to SBUF immediately (frees
                    # the 2-deep psum ring for the next half), 1/den rows ->
                    # DRAM-bounce broadcast, normalize from the SBUF copies;
                    # head-odd is DMA-shifted (sbuf->sbuf) to lanes 64:128
                    ouA = p2.tile([P, TB], F32, tag="ouA", bufs=2, name="ouA")
                    ouB = p2.tile([P, TB], F32R, tag="ouB", bufs=2, name="ouB")
                    den_inv = p2.tile([P, 2, TB], F32, tag="den_inv",
                                      bufs=2, name="den_inv")
                    last = hi == len(halves) - 1
                    if last:
                        # no next half to feed: reciprocals straight from
                        # PSUM, ahead of the staging copies, shorten the
                        # tail's den -> rep -> normalize chain
                        for hh, pv in ((0, pvA), (1, pvB)):
                            nc.vector.reciprocal(
                                out=den_inv[DK:DK + 1, hh, :],
                                in_=pv[DK:DK + 1, :],
                            )
                            nc.sync.dma_start(
                                out=den_dram[2 * tb + m, hh:hh + 1, :],
                                in_=den_inv[DK:DK + 1, hh, :],
                            )
                    nc.vector.tensor_copy(out=ouA[0:DK + 1, :], in_=pvA[0:DK + 1, :])
                    nc.vector.tensor_copy(out=ouB[0:DK + 1, :], in_=pvB[0:DK + 1, :])
                    if not last:
                        # mid-halves read the SBUF staging copy so the pv
                        # bank is released by the copy alone (2-deep ring)
                        for hh, ou in ((0, ouA), (1, ouB)):
                            nc.vector.reciprocal(
                                out=den_inv[DK:DK + 1, hh, :],
                                in_=ou[DK:DK + 1, :],
                            )
                            nc.sync.dma_start(
                                out=den_dram[2 * tb + m, hh:hh + 1, :],
                                in_=den_inv[DK:DK + 1, hh, :],
                            )
                    rep = p2.tile([P, TB], F32, tag="rep", bufs=2, name="rep")
                    for hh in range(2):
                        nc.sync.dma_start(
                            out=rep[DK * hh:DK * hh + DK, :],
                            in_=den_dram[2 * tb + m, hh:hh + 1, :].to_broadcast([DK, TB]),
                        )
                    ou2 = p2.tile([P, TB], F32R, tag="ou2", bufs=2, name="ou2")
                    nc.sync.dma_start(out=ou2[DK:P, :], in_=ouB[0:DK, :])
                    nc.vector.tensor_mul(
                        outT[0:DK, m, t0:t0 + TB], ouA[0:DK, :], rep[0:DK, :]
                    )
                    nc.vector.tensor_mul(
                        outT[DK:P, m, t0:t0 + TB], ou2[DK:P, :], rep[DK:P, :]
                    )
                    if m == 1:
                        pending = [(tt, fb)
                                   for tt in range(tb * (TB // P), (tb + 1) * (TB // P))
                                   for fb in range(2)]
                # tail units: the pv ring, kq and score banks are free now --
                # spread across 6 banks so the units pipeline instead of
                # serializing; ACT is idle after the final exp, so it does
                # the PSUM->SBUF copies
                banks = [
                    lambda: ps_y.tile([P, 512], F32, tag="y", name="y_ps"),
                    lambda: ps_pv.tile([P, TB], F32, tag="pv", name="y_ps"),
                    lambda: ps_pv.tile([P, TB], F32, tag="pv", name="y_ps"),
                    lambda: ps_pv.tile([P, 512], F32, tag="kq", bufs=1, name="y_ps"),
                    lambda: ps_sc.tile([P, 2, TB], F32, tag="sc", name="y_ps")[:, 0, :],
                    lambda: ps_sc.tile([P, 2, TB], F32, tag="sc", name="y_ps")[:, 0, :],
                ]
                for i, u in enumerate(pending):
                    y_unit(*u, mk=banks[i % 6], act_copy=True)
    _split_multi_waits(nc)
    return nc


def _shard_inputs(x, w_q, b_q, w_k, b_k, w_v, b_v, w_o, b_o):
    import ml_dtypes
    bf16 = ml_dtypes.bfloat16
    in_maps = []
    for c in range(N_CORES):
        b, g = c // 4, c % 4
        sl = slice(g * E, (g + 1) * E)
        in_maps.append({
            "xT": np.ascontiguousarray(x[b].T).astype(bf16),
            "wqT": np.ascontiguousarray(w_q[sl, :].T).astype(bf16),
            "wkT": np.ascontiguousarray(w_k[sl, :].T).astype(bf16),
            "wvT": np.ascontiguousarray(w_v[sl, :].T).astype(bf16),
            "wo_sh": np.ascontiguousarray(w_o[:, sl].T, dtype=np.float32),
            "bq2": np.ascontiguousarray(b_q[sl].reshape(2, P).T, dtype=np.float32),
            "bk2": np.ascontiguousarray(b_k[sl].reshape(2, P).T, dtype=np.float32),
        })
    return in_maps


_NC_CACHE = {}


def kernel(x, w_q, b_q, w_k, b_k, w_v, b_v, w_o, b_o, _trace=False):
    x = np.asarray(x, dtype=np.float32)
    B, T, _ = x.shape
    args = [np.asarray(a, dtype=np.float32)
            for a in (w_q, b_q, w_k, b_k, w_v, b_v, w_o, b_o)]
    w_q, b_q, w_k, b_k, w_v, b_v, w_o, b_o = args

    if T not in _NC_CACHE:
        _NC_CACHE[T] = build_nc(T=T)
    nc = _NC_CACHE[T]
    in_maps = _shard_inputs(x, w_q, b_q, w_k, b_k, w_v, b_v, w_o, b_o)
    res = run_bass_kernel_spmd(nc, in_maps, list(range(N_CORES)), trace=_trace)

    y = np.zeros((B, T, D), dtype=np.float32)
    for c in range(N_CORES):
        y[c // 4] += np.asarray(res.results[c]["y"], dtype=np.float32)
    fold = b_v @ w_o.T + b_o
    y += fold[None, None, :]
    if _trace:
        return y, res
    return y


# revision 42
# speedup vs baseline: 1.0139x; 1.0095x over previous
"""Multi-head attention (B=2, T=2048, D=1024, H=16) on 8 NeuronCores.

Sharding: core c handles batch b=c//4 and head-group g=c%4 (4 heads = 256
of the 1024 e-dims). QKV weights are column-sharded, w_o row-sharded.
The host transposes x and the weight shards (cast to bf16) so every device
matmul has its contraction dim on partitions with no on-device transposes.
Each core returns a [T, D] partial of the output projection; the host sums
the 4 partials per batch (the TP all-reduce) and folds in b_v @ w_o^T + b_o.

Device algorithm (per core):
  Minimal prologue: only K-chunk0/Q-chunk0 projections run before the
  attention stream starts (~10us); the remaining K/Q projection chunks are
  DRIPPED into the s-loops through a dedicated 1-bank psum slot ("kq"),
  paced so each chunk lands just before the scores that need it. x stays
  resident in SBUF (bf16) and the V projection streams from it inside the
  first half-block's s-loop.
  Each 512-wide t-block is split into two HEAD-PAIR halves (half m covers
  heads 2m, 2m+1 -> outT[:, m, :]). Per half, per 128-wide s-tile:
    scores^T psum [s, 2, t] via 2 matmuls (head hh at KT/QT partition base
    64*hh), prefetched one iteration ahead (incl. across half boundaries)
    so the exp stream never waits on PE; ONE Exp activation (free-1024) ->
    pT bf16; P@V via per-head [128, 65] lhsT = [V_h | ones]: psum row 64
    accumulates the softmax denominator for free (NO separate denominator
    matmuls), trailing exp by two s-tiles.
  Tail per half: both pv banks staged to SBUF immediately (frees the
  2-deep pv ring for the next half), 1/den -> DRAM-bounce broadcast,
  head-odd DMA-shifted (sbuf->sbuf) to lanes 64:128, normalize into outT.
  Output projection: per [128 t, 512 f] block, ONE accumulation group of
  2 full-128-contraction matmuls (head pair fused via outT/wo_sb layout),
  dripped into the NEXT t-block's ACT-bound s-loops.
  PSUM: scores 2x[128,2,512] (4) + pv ring 2 + kq 1 + y/V 1 = 8 banks.
"""

import sys
from contextlib import ExitStack

import numpy as np

try:
    import concourse.bass as bass
except ImportError:  # pragma: no cover
    sys.path.insert(0, "/opt/trn_rl_repo")
    import concourse.bass as bass

import concourse.tile as tile
from concourse import mybir
from concourse.bass_utils import run_bass_kernel_spmd

F32 = mybir.dt.float32
F32R = mybir.dt.float32r
BF16 = mybir.dt.bfloat16

D = 1024
H = 16
DK = 64
E = 256  # per-core out-dim of the head group (4 heads x 64)
P = 128
N_CORES = 8


def _split_multi_waits(nc):
    """This container's walrus encodes at most ONE sync-wait per instruction
    ("Too many sync wait commands" in codegen otherwise). Tile attaches
    multi-sem waits to instructions; hoist all but the last wait onto
    standalone single-wait EventSemaphore instructions inserted just before,
    on the same engine — semantically identical (engine stalls in order)."""
    n = 0
    for fn in nc.m.functions:
        for bb in fn.blocks:
            il = bb.instructions
            i = 0
            while i < len(il):
                ins = il[i]
                si = ins.sync_info
                if si is not None and si.on_wait and len(si.on_wait) > 1:
                    waits = list(si.on_wait)
                    for k, w in enumerate(waits[:-1]):
                        ev = mybir.InstEventSemaphore(
                            name=f"{ins.name}_w{k}", ins=[], outs=[],
                            sync_info=mybir.SyncInfo(on_wait=[w], on_update=[]),
                        )
                        ev.engine = ins.engine
                        nc.register_instruction(ev)
                        il.insert(i, ev)
                        i += 1
                        n += 1
                    si.on_wait = waits[-1:]
                i += 1
    return n


def build_nc(T=2048, TB=512):
    """Build the SPMD Bass program (identical on all 8 cores)."""
    NT = T // P       # number of 128-wide s-tiles / t-tiles
    NTB = T // TB     # number of t-blocks in phase 2
    NPB = T // 512    # number of 512-wide t-chunks for projections

    nc = bass.Bass()

    xT_d = nc.dram_tensor("xT", [D, T], BF16, kind="ExternalInput")
    wqT_d = nc.dram_tensor("wqT", [D, E], BF16, kind="ExternalInput")
    wkT_d = nc.dram_tensor("wkT", [D, E], BF16, kind="ExternalInput")
    wvT_d = nc.dram_tensor("wvT", [D, E], BF16, kind="ExternalInput")
    wo_d = nc.dram_tensor("wo_sh", [E, D], F32R, kind="ExternalInput")
    bq_d = nc.dram_tensor("bq2", [P, 2], F32, kind="ExternalInput")
    bk_d = nc.dram_tensor("bk2", [P, 2], F32, kind="ExternalInput")
    y_d = nc.dram_tensor("y", [T, D], BF16, kind="ExternalOutput")
    den_dram = nc.dram_tensor("den_scratch", [NTB * 2, 2, TB], F32)

    with tile.TileContext(nc) as tc:
        with tc.tile_pool(name="const", bufs=1) as const:
            QT = const.tile([P, 2, T], F32R)       # [e%128, pair, t]
            KT = const.tile([P, 2, T], F32R)
            V = const.tile([P, NT, 4, DK + 1], BF16)  # [s%128, s//128, h, dk|1]
            pT = const.tile([P, NT, 2, TB], BF16)  # exp(scores^T) of one half
            outT = const.tile([P, 2, T], F32R)     # normalized (attn @ V)^T
            wo_sb = const.tile([P, 2, D], F32R)
            bq_sb = const.tile([P, 2], F32)
            bk_sb = const.tile([P, 2], F32)

            # ones column: pv psum row 64 accumulates the softmax denominator
            nc.vector.memset(V[:, :, :, DK:DK + 1], 1.0)

            # x + projection weights stay resident in SBUF (bf16, ~44KB)
            xT_sb = const.tile([P, 8, T], BF16)
            wv_sb = const.tile([P, 8, E], BF16)
            wk_sb = const.tile([P, 8, E], BF16)
            wq_sb = const.tile([P, 8, E], BF16)

            # DMA order = need order: wk+x0 gate the prologue, wq gates
            # Q0, wv the in-loop V projection, x1-3 dripped K chunks
            nc.sync.dma_start(out=wk_sb, in_=wkT_d[:].rearrange("(dt p) e -> p dt e", p=P))
            nc.sync.dma_start(out=wq_sb, in_=wqT_d[:].rearrange("(dt p) e -> p dt e", p=P))
            for dt in range(8):
                nc.sync.dma_start(
                    out=xT_sb[:, dt, 0:512], in_=xT_d[dt * P:(dt + 1) * P, 0:512]
                )
            nc.sync.dma_start(out=bq_sb, in_=bq_d[:])
            nc.sync.dma_start(out=bk_sb, in_=bk_d[:])
            nc.sync.dma_start(out=wv_sb, in_=wvT_d[:].rearrange("(dt p) e -> p dt e", p=P))
            for t4 in range(1, NPB):
                for dt in range(8):
                    nc.sync.dma_start(
                        out=xT_sb[:, dt, t4 * 512:(t4 + 1) * 512],
                        in_=xT_d[dt * P:(dt + 1) * P, t4 * 512:(t4 + 1) * 512],
                    )
            # wo is not needed until the first output projection
            nc.sync.dma_start(out=wo_sb, in_=wo_d[:].rearrange("(m p) f -> p m f", p=P))

            # ---- prologue: K chunk0 + Q chunk0 only (via a scoped pool) ----
            with tc.tile_pool(name="ps1", bufs=4, space="PSUM") as ps1:
                for w_sb, dst, b_sb, t4, em in (
                    (wk_sb, KT, bk_sb, 0, 0),
                    (wq_sb, QT, bq_sb, 0, 0),
                    (wq_sb, QT, bq_sb, 0, 1),
                    (wk_sb, KT, bk_sb, 0, 1),
                ):
                    ps = ps1.tile([P, 512], F32, tag="proj", name="proj_ps")
                    for dt in range(8):
                        nc.tensor.matmul(
                            ps,
                            lhsT=w_sb[:, dt, em * P:(em + 1) * P],
                            rhs=xT_sb[:, dt, t4 * 512:(t4 + 1) * 512],
                            start=(dt == 0),
                            stop=(dt == 7),
                        )
                    nc.vector.tensor_scalar_add(
                        out=dst[:, em, t4 * 512:(t4 + 1) * 512],
                        in0=ps,
                        scalar1=b_sb[:, em:em + 1],
                    )

            # -------- phase 2: attention + fused output projection --------
            with (
                tc.tile_pool(name="p2", bufs=1) as p2,
                tc.tile_pool(name="ps_sc", bufs=2, space="PSUM") as ps_sc,
                tc.tile_pool(name="ps_pv", bufs=2, space="PSUM") as ps_pv,
                tc.tile_pool(name="ps_y", bufs=1, space="PSUM") as ps_y,
            ):
                # ---- dripped projection chunks (the "kq" 1-bank slot) ----
                # each group = one [128e, 512t] projection accumulation,
                # emitted in slices of `dts` so it hides in s-loop slack
                class Group:
                    def __init__(self, w_sb, dst, b_sb, t4, em):
                        self.w_sb, self.dst, self.b_sb = w_sb, dst, b_sb
                        self.t4, self.em = t4, em
                        self.ps = None

                    def emit(self, dts, last):
                        if self.ps is None:
                            self.ps = ps_pv.tile([P, 512], F32, tag="kq",
                                                 bufs=1, name="kq_ps")
                        for dt in dts:
                            nc.tensor.matmul(
                                self.ps,
                                lhsT=self.w_sb[:, dt, self.em * P:(self.em + 1) * P],
                                rhs=xT_sb[:, dt, self.t4 * 512:(self.t4 + 1) * 512],
                                start=(dt == 0),
                                stop=(dt == 7),
                            )
                        if last:
                            nc.vector.tensor_scalar_add(
                                out=self.dst[:, self.em,
                                             self.t4 * 512:(self.t4 + 1) * 512],
                                in0=self.ps,
                                scalar1=self.b_sb[:, self.em:self.em + 1],
                            )

                def KG(t4, em):
                    return Group(wk_sb, KT, bk_sb, t4, em)

                def QG(t4, em):
                    return Group(wq_sb, QT, bq_sb, t4, em)

                # drip_plan[hi][st] = (group, dts, last)
                drip_plan = {}

                def plan(hi, g, st0):
                    drip_plan.setdefault(hi, {})[st0] = (g, range(0, 4), False)
                    drip_plan[hi][st0 + 1] = (g, range(4, 8), True)

                # half 0 absorbs K chunks 1-3 (em0 just ahead of its own
                # scores, em1 before half 1 needs them)
                plan(0, KG(1, 0), 1)
                plan(0, KG(1, 1), 3)
                plan(0, KG(2, 0), 5)
                plan(0, KG(2, 1), 7)
                plan(0, KG(3, 0), 9)
                plan(0, KG(3, 1), 11)
                # Q chunk c, pair em is first needed by half (2c + em)'s
                # scores, prefetched at the END of half (2c + em - 1): one
                # group per half, each landing just in time
                plan(1, QG(1, 0), 2)
                plan(2, QG(1, 1), 2)
                plan(3, QG(2, 0), 2)
                plan(4, QG(2, 1), 2)
                plan(5, QG(3, 0), 2)
                plan(6, QG(3, 1), 2)

                def y_unit(tt, fb, mk=None, act_copy=False):
                    # output projection for one [128 t, 512 f] block: one
                    # accumulation group of 2 full-128-contraction matmuls
                    # (head pair via outT/wo_sb layout), copy to SBUF, DMA
                    f0 = fb * 512
                    if mk is None:
                        yps = ps_y.tile([P, 512], F32, tag="y", name="y_ps")
                    else:
                        yps = mk()
                    for m2 in range(2):
                        nc.tensor.matmul(
                            yps,
                            lhsT=outT[:, m2, tt * P:(tt + 1) * P],
                            rhs=wo_sb[:, m2, f0:f0 + 512],
                            start=(m2 == 0),
                            stop=(m2 == 1),
                            skip_group_check=True,
                        )
                    ysb = p2.tile([P, 512], BF16, tag="ysb", bufs=3, name="ysb")
                    if act_copy:
                        nc.scalar.copy(out=ysb, in_=yps)
                    else:
                        nc.vector.tensor_copy(out=ysb, in_=yps)
                    nc.sync.dma_start(
                        out=y_d[tt * P:(tt + 1) * P, f0:f0 + 512], in_=ysb
                    )

                def emit_scores(m, t0, st):
                    sc = ps_sc.tile([P, 2, TB], F32, tag="sc", name="sc_ps")
                    for hh in range(2):
                        p0 = DK * hh
                        nc.tensor.matmul(
                            sc[:, hh, :],
                            lhsT=KT[p0:p0 + DK, m, st * P:(st + 1) * P],
                            rhs=QT[p0:p0 + DK, m, t0:t0 + TB],
                            start=True,
                            stop=True,
                        )
                    return sc

                halves = [(tb, m) for tb in range(NTB) for m in range(2)]
                pending = []  # deferred y-units of the previous t-block
                sc_cur = emit_scores(halves[0][1], halves[0][0] * TB, 0)
                for hi, (tb, m) in enumerate(halves):
                    t0 = tb * TB
                    pvA = ps_pv.tile([P, TB], F32, tag="pv", name="pvA")
                    pvB = ps_pv.tile([P, TB], F32, tag="pv", name="pvB")

                    def pv_dn(st):
                        for hh, pv in ((0, pvA), (1, pvB)):
                            nc.tensor.matmul(
                                pv[0:DK + 1, :],
                                lhsT=V[:, st, 2 * m + hh, :],
                                rhs=pT[:, st, hh, :],
                                start=(st == 0),
                                stop=(st == NT - 1),
                                skip_group_check=True,
                            )

                    for st in range(NT):
                        # scores are emitted one iteration AHEAD (incl.
                        # across half boundaries) so the exp stream never
                        # waits on PE
                        if st + 1 < NT:
                            sc_nxt = emit_scores(m, t0, st + 1)
                        elif hi + 1 < len(halves):
                            tb2, m2 = halves[hi + 1]
                            sc_nxt = emit_scores(m2, tb2 * TB, 0)
                        else:
                            sc_nxt = None
                        if hi == 0:
                            # V projection from the resident bf16 x, one
                            # s-chunk per iteration, psum via the y bank
                            vps = ps_y.tile([P, 512], F32, tag="y", name="v_ps")
                            for dt in range(8):
                                nc.tensor.matmul(
                                    vps[:, :E],
                                    lhsT=xT_sb[:, dt, st * P:(st + 1) * P],
                                    rhs=wv_sb[:, dt, :],
                                    start=(dt == 0),
                                    stop=(dt == 7),
                                )
                            nc.vector.tensor_copy(
                                out=V[:, st, :, 0:DK], in_=vps[:, :E]
                            )
                        # software pipeline (depth 2): P@V trails exp by two
                        # s-tiles so a new half's first pv matmul never waits
                        # on the previous half's pv banks still draining
                        if st > 1:
                            pv_dn(st - 2)
                        # dripped projection slice for this iteration
                        if hi in drip_plan and st in drip_plan[hi]:
                            g, dts, last = drip_plan[hi][st]
                            g.emit(dts, last)
                        nc.scalar.activation(
                            out=pT[:, st, :, :],
                            in_=sc_cur,
                            func=mybir.ActivationFunctionType.Exp,
                            scale=0.125,
                        )
                        sc_cur = sc_nxt
                        # drip the previous t-block's output projection into
                        # this (ACT-bound) s-loop
                        if pending and st in (5, 8, 11, 14):
                            y_unit(*pending.pop(0))
                    pv_dn(NT - 2)
                    pv_dn(NT - 1)
                    # tail: stage both pv banks to SBUF immediately (frees
                    # the 2-deep psum ring for the next half), 1/den rows ->
                    # DRAM-bounce broadcast, normalize from the SBUF copies;
                    # head-odd is DMA-shifted (sbuf->sbuf) to lanes 64:128
                    ouA = p2.tile([P, TB], F32, tag="ouA", bufs=2, name="ouA")
                    ouB = p2.tile([P, TB], F32R, tag="ouB", bufs=2, name="ouB")
                    den_inv = p2.tile([P, 2, TB], F32, tag="den_inv",
                                      bufs=2, name="den_inv")
                    last = hi == len(halves) - 1
                    if last:
                        # no next half to feed: reciprocals straight from
                        # PSUM, ahead of the staging copies, shorten the
                        # tail's den -> rep -> normalize chain
                        for hh, pv in ((0, pvA), (1, pvB)):
                            nc.vector.reciprocal(
                                out=den_inv[DK:DK + 1, hh, :],
                                in_=pv[DK:DK + 1, :],
                            )
                            nc.sync.dma_start(
                                out=den_dram[2 * tb + m, hh:hh + 1, :],
                                in_=den_inv[DK:DK + 1, hh, :],
                            )
                    nc.vector.tensor_copy(out=ouA[0:DK + 1, :], in_=pvA[0:DK + 1, :])
                    nc.vector.tensor_copy(out=ouB[0:DK + 1, :], in_=pvB[0:DK + 1, :])
                    if not last:
                        # mid-halves read the SBUF staging copy so the pv
                        # bank is released by the copy alone (2-deep ring)
                        for hh, ou in ((0, ouA), (1, ouB)):
                            nc.vector.reciprocal(
                                out=den_inv[DK:DK + 1, hh, :],
                                in_=ou[DK:DK + 1, :],
                            )
                            nc.sync.dma_start(
                                out=den_dram[2 * tb + m, hh:hh + 1, :],
                                in_=den_inv[DK:DK + 1, hh, :],
                            )
                    rep = p2.tile([P, TB], F32, tag="rep", bufs=2, name="rep")
                    for hh in range(2):
                        nc.sync.dma_start(
                            out=rep[DK * hh:DK * hh + DK, :],
                            in_=den_dram[2 * tb + m, hh:hh + 1, :].to_broadcast([DK, TB]),
                        )
                    ou2 = p2.tile([P, TB], F32R, tag="ou2", bufs=2, name="ou2")
                    nc.sync.dma_start(out=ou2[DK:P, :], in_=ouB[0:DK, :])
                    nc.vector.tensor_mul(
                        outT[0:DK, m, t0:t0 + TB], ouA[0:DK, :], rep[0:DK, :]
                    )
                    nc.vector.tensor_mul(
                        outT[DK:P, m, t0:t0 + TB], ou2[DK:P, :], rep[DK:P, :]
                    )
                    if m == 1:
                        pending = [(tt, fb)
                                   for tt in range(tb * (TB // P), (tb + 1) * (TB // P))
                                   for fb in range(2)]
                # tail units: the pv ring, kq and score banks are free now --
                # spread across 6 banks so the units pipeline instead of
                # serializing; ACT is idle after the final exp, so it does
                # the PSUM->SBUF copies
                banks = [
                    lambda: ps_y.tile([P, 512], F32, tag="y", name="y_ps"),
                    lambda: ps_pv.tile([P, TB], F32, tag="pv", name="y_ps"),
                    lambda: ps_pv.tile([P, TB], F32, tag="pv", name="y_ps"),
                    lambda: ps_pv.tile([P, 512], F32, tag="kq", bufs=1, name="y_ps"),
                    lambda: ps_sc.tile([P, 2, TB], F32, tag="sc", name="y_ps")[:, 0, :],
                    lambda: ps_sc.tile([P, 2, TB], F32, tag="sc", name="y_ps")[:, 0, :],
                ]
                for i, u in enumerate(pending):
                    y_unit(*u, mk=banks[i % 6], act_copy=True)
    _split_multi_waits(nc)
    return nc


def _shard_inputs(x, w_q, b_q, w_k, b_k, w_v, b_v, w_o, b_o):
    import ml_dtypes
    bf16 = ml_dtypes.bfloat16
    in_maps = []
    for c in range(N_CORES):
        b, g = c // 4, c % 4
        sl = slice(g * E, (g + 1) * E)
        in_maps.append({
            "xT": np.ascontiguousarray(x[b].T).astype(bf16),
            "wqT": np.ascontiguousarray(w_q[sl, :].T).astype(bf16),
            "wkT": np.ascontiguousarray(w_k[sl, :].T).astype(bf16),
            "wvT": np.ascontiguousarray(w_v[sl, :].T).astype(bf16),
            "wo_sh": np.ascontiguousarray(w_o[:, sl].T, dtype=np.float32),
            "bq2": np.ascontiguousarray(b_q[sl].reshape(2, P).T, dtype=np.float32),
            "bk2": np.ascontiguousarray(b_k[sl].reshape(2, P).T, dtype=np.float32),
        })
    return in_maps


_NC_CACHE = {}


def kernel(x, w_q, b_q, w_k, b_k, w_v, b_v, w_o, b_o, _trace=False):
    x = np.asarray(x, dtype=np.float32)
    B, T, _ = x.shape
    args = [np.asarray(a, dtype=np.float32)
            for a in (w_q, b_q, w_k, b_k, w_v, b_v, w_o, b_o)]
    w_q, b_q, w_k, b_k, w_v, b_v, w_o, b_o = args

    if T not in _NC_CACHE:
        _NC_CACHE[T] = build_nc(T=T)
    nc = _NC_CACHE[T]
    in_maps = _shard_inputs(x, w_q, b_q, w_k, b_k, w_v, b_v, w_o, b_o)
    res = run_bass_kernel_spmd(nc, in_maps, list(range(N_CORES)), trace=_trace)

    y = np.zeros((B, T, D), dtype=np.float32)
    for c in range(N_CORES):
        y[c // 4] += np.asarray(res.results[c]["y"], dtype=np.float32)
    fold = b_v @ w_o.T + b_o
    y += fold[None, None, :]
    if _trace:
        return y, res
    return y


# revision 47
# speedup vs baseline: 1.0285x; 1.0143x over previous
"""Multi-head attention (B=2, T=2048, D=1024, H=16) on 8 NeuronCores.

Sharding: core c handles batch b=c//4 and head-group g=c%4 (4 heads = 256
of the 1024 e-dims). QKV weights are column-sharded, w_o row-sharded.
The host transposes x and the weight shards (cast to bf16) so every device
matmul has its contraction dim on partitions with no on-device transposes.
Each core returns a [T, D] partial of the output projection; the host sums
the 4 partials per batch (the TP all-reduce) and folds in b_v @ w_o^T + b_o.

Device algorithm (per core):
  Minimal prologue: only K-chunk0/Q-chunk0 projections run before the
  attention stream starts (~10us); the remaining K/Q projection chunks are
  DRIPPED into the s-loops through a dedicated 1-bank psum slot ("kq"),
  paced so each chunk lands just before the scores that need it. x stays
  resident in SBUF (bf16) and the V projection streams from it inside the
  first half-block's s-loop.
  Each 512-wide t-block is split into two HEAD-PAIR halves (half m covers
  heads 2m, 2m+1 -> outT[:, m, :]). Per half, per 128-wide s-tile:
    scores^T psum [s, 2, t] via 2 matmuls (head hh at KT/QT partition base
    64*hh), prefetched one iteration ahead (incl. across half boundaries)
    so the exp stream never waits on PE; ONE Exp activation (free-1024) ->
    pT bf16; P@V via per-head [128, 65] lhsT = [V_h | ones]: psum row 64
    accumulates the softmax denominator for free (NO separate denominator
    matmuls), trailing exp by two s-tiles.
  Tail per half: both pv banks staged to SBUF immediately (frees the
  2-deep pv ring for the next half), 1/den -> DRAM-bounce broadcast,
  head-odd DMA-shifted (sbuf->sbuf) to lanes 64:128, normalize into outT.
  Output projection: per [128 t, 512 f] block, ONE accumulation group of
  2 full-128-contraction matmuls (head pair fused via outT/wo_sb layout),
  dripped into the NEXT t-block's ACT-bound s-loops.
  PSUM: scores 2x[128,2,512] (4) + pv ring 2 + kq 1 + y/V 1 = 8 banks.
"""

import sys
from contextlib import ExitStack

import numpy as np

try:
    import concourse.bass as bass
except ImportError:  # pragma: no cover
    sys.path.insert(0, "/opt/trn_rl_repo")
    import concourse.bass as bass

import concourse.tile as tile
from concourse import mybir
from concourse.bass_utils import run_bass_kernel_spmd

F32 = mybir.dt.float32
F32R = mybir.dt.float32r
BF16 = mybir.dt.bfloat16

D = 1024
H = 16
DK = 64
E = 256  # per-core out-dim of the head group (4 heads x 64)
P = 128
N_CORES = 8


def _split_multi_waits(nc):
    """This container's walrus encodes at most ONE sync-wait per instruction
    ("Too many sync wait commands" in codegen otherwise). Tile attaches
    multi-sem waits to instructions; hoist all but the last wait onto
    standalone single-wait EventSemaphore instructions inserted just before,
    on the same engine — semantically identical (engine stalls in order)."""
    n = 0
    for fn in nc.m.functions:
        for bb in fn.blocks:
            il = bb.instructions
            i = 0
            while i < len(il):
                ins = il[i]
                si = ins.sync_info
                if si is not None and si.on_wait and len(si.on_wait) > 1:
                    waits = list(si.on_wait)
                    for k, w in enumerate(waits[:-1]):
                        ev = mybir.InstEventSemaphore(
                            name=f"{ins.name}_w{k}", ins=[], outs=[],
                            sync_info=mybir.SyncInfo(on_wait=[w], on_update=[]),
                        )
                        ev.engine = ins.engine
                        nc.register_instruction(ev)
                        il.insert(i, ev)
                        i += 1
                        n += 1
                    si.on_wait = waits[-1:]
                i += 1
    return n


def build_nc(T=2048, TB=512):
    """Build the SPMD Bass program (identical on all 8 cores)."""
    NT = T // P       # number of 128-wide s-tiles / t-tiles
    NTB = T // TB     # number of t-blocks in phase 2
    NPB = T // 512    # number of 512-wide t-chunks for projections

    nc = bass.Bass()

    xT_d = nc.dram_tensor("xT", [D, T], BF16, kind="ExternalInput")
    wqT_d = nc.dram_tensor("wqT", [D, E], BF16, kind="ExternalInput")
    wkT_d = nc.dram_tensor("wkT", [D, E], BF16, kind="ExternalInput")
    wvT_d = nc.dram_tensor("wvT", [D, E], BF16, kind="ExternalInput")
    wo_d = nc.dram_tensor("wo_sh", [E, D], F32R, kind="ExternalInput")
    bq_d = nc.dram_tensor("bq2", [P, 2], F32, kind="ExternalInput")
    bk_d = nc.dram_tensor("bk2", [P, 2], F32, kind="ExternalInput")
    y_d = nc.dram_tensor("y", [T, D], BF16, kind="ExternalOutput")
    den_dram = nc.dram_tensor("den_scratch", [NTB * 2, 2, TB], F32)

    with tile.TileContext(nc) as tc:
        with tc.tile_pool(name="const", bufs=1) as const:
            QT = const.tile([P, 2, T], F32R)       # [e%128, pair, t]
            KT = const.tile([P, 2, T], F32R)
            V = const.tile([P, NT, 4, DK + 1], BF16)  # [s%128, s//128, h, dk|1]
            pT = const.tile([P, NT, 2, TB], BF16)  # exp(scores^T) of one half
            outT = const.tile([P, 2, T], F32R)     # normalized (attn @ V)^T
            wo_sb = const.tile([P, 2, D], F32R)
            bq_sb = const.tile([P, 2], F32)
            bk_sb = const.tile([P, 2], F32)

            # ones column: pv psum row 64 accumulates the softmax denominator
            nc.vector.memset(V[:, :, :, DK:DK + 1], 1.0)

            # x + projection weights stay resident in SBUF (bf16, ~44KB)
            xT_sb = const.tile([P, 8, T], BF16)
            wv_sb = const.tile([P, 8, E], BF16)
            wk_sb = const.tile([P, 8, E], BF16)
            wq_sb = const.tile([P, 8, E], BF16)

            # DMA order = need order: wk+x0 gate the prologue, wq gates
            # Q0, wv the in-loop V projection, x1-3 dripped K chunks
            nc.sync.dma_start(out=wk_sb, in_=wkT_d[:].rearrange("(dt p) e -> p dt e", p=P))
            nc.sync.dma_start(out=wq_sb, in_=wqT_d[:].rearrange("(dt p) e -> p dt e", p=P))
            for dt in range(8):
                nc.sync.dma_start(
                    out=xT_sb[:, dt, 0:512], in_=xT_d[dt * P:(dt + 1) * P, 0:512]
                )
            nc.sync.dma_start(out=bq_sb, in_=bq_d[:])
            nc.sync.dma_start(out=bk_sb, in_=bk_d[:])
            nc.sync.dma_start(out=wv_sb, in_=wvT_d[:].rearrange("(dt p) e -> p dt e", p=P))
            for t4 in range(1, NPB):
                for dt in range(8):
                    nc.sync.dma_start(
                        out=xT_sb[:, dt, t4 * 512:(t4 + 1) * 512],
                        in_=xT_d[dt * P:(dt + 1) * P, t4 * 512:(t4 + 1) * 512],
                    )
            # wo is not needed until the first output projection
            nc.sync.dma_start(out=wo_sb, in_=wo_d[:].rearrange("(m p) f -> p m f", p=P))

            # ---- prologue: K chunk0 + Q chunk0 only (via a scoped pool) ----
            with tc.tile_pool(name="ps1", bufs=4, space="PSUM") as ps1:
                for w_sb, dst, b_sb, t4, em in (
                    (wk_sb, KT, bk_sb, 0, 0),
                    (wq_sb, QT, bq_sb, 0, 0),
                    (wq_sb, QT, bq_sb, 0, 1),
                    (wk_sb, KT, bk_sb, 0, 1),
                ):
                    ps = ps1.tile([P, 512], F32, tag="proj", name="proj_ps")
                    for dt in range(8):
                        nc.tensor.matmul(
                            ps,
                            lhsT=w_sb[:, dt, em * P:(em + 1) * P],
                            rhs=xT_sb[:, dt, t4 * 512:(t4 + 1) * 512],
                            start=(dt == 0),
                            stop=(dt == 7),
                        )
                    nc.vector.tensor_scalar_add(
                        out=dst[:, em, t4 * 512:(t4 + 1) * 512],
                        in0=ps,
                        scalar1=b_sb[:, em:em + 1],
                    )

            # -------- phase 2: attention + fused output projection --------
            with (
                tc.tile_pool(name="p2", bufs=1) as p2,
                tc.tile_pool(name="ps_sc", bufs=2, space="PSUM") as ps_sc,
                tc.tile_pool(name="ps_pv", bufs=2, space="PSUM") as ps_pv,
                tc.tile_pool(name="ps_y", bufs=1, space="PSUM") as ps_y,
            ):
                # ---- dripped projection chunks (the "kq" 1-bank slot) ----
                # each group = one [128e, 512t] projection accumulation,
                # emitted in slices of `dts` so it hides in s-loop slack
                class Group:
                    def __init__(self, w_sb, dst, b_sb, t4, em):
                        self.w_sb, self.dst, self.b_sb = w_sb, dst, b_sb
                        self.t4, self.em = t4, em
                        self.ps = None

                    def emit(self, dts, last):
                        if self.ps is None:
                            self.ps = ps_pv.tile([P, 512], F32, tag="kq",
                                                 bufs=1, name="kq_ps")
                        for dt in dts:
                            nc.tensor.matmul(
                                self.ps,
                                lhsT=self.w_sb[:, dt, self.em * P:(self.em + 1) * P],
                                rhs=xT_sb[:, dt, self.t4 * 512:(self.t4 + 1) * 512],
                                start=(dt == 0),
                                stop=(dt == 7),
                            )
                        if last:
                            nc.vector.tensor_scalar_add(
                                out=self.dst[:, self.em,
                                             self.t4 * 512:(self.t4 + 1) * 512],
                                in0=self.ps,
                                scalar1=self.b_sb[:, self.em:self.em + 1],
                            )

                def KG(t4, em):
                    return Group(wk_sb, KT, bk_sb, t4, em)

                def QG(t4, em):
                    return Group(wq_sb, QT, bq_sb, t4, em)

                # drip_plan[hi][st] = (group, dts, last)
                drip_plan = {}

                def plan(hi, g, st0, per=4):
                    sched = drip_plan.setdefault(hi, {})
                    for i in range(0, 8, per):
                        sched[st0 + i // per] = (g, range(i, i + per),
                                                 i + per == 8)

                # half 0 absorbs K chunks 1-3 (em0 just ahead of its own
                # scores, em1 before half 1 needs them); half 0 is PE-bound
                # anyway, so 4 d-tiles per iteration
                plan(0, KG(1, 0), 1)
                plan(0, KG(1, 1), 3)
                plan(0, KG(2, 0), 5)
                plan(0, KG(2, 1), 7)
                plan(0, KG(3, 0), 9)
                plan(0, KG(3, 1), 11)
                # Q chunk c, pair em is first needed by half (2c + em)'s
                # scores, prefetched at the END of half (2c + em - 1): one
                # group per half at ONE d-tile per iteration -- 213ns/iter
                # sits just above the 186ns s-loop slack, so the exp stream
                # is essentially undisturbed
                plan(1, QG(1, 0), 2, per=1)
                plan(2, QG(1, 1), 2, per=1)
                plan(3, QG(2, 0), 2, per=1)
                plan(4, QG(2, 1), 2, per=1)
                plan(5, QG(3, 0), 2, per=1)
                plan(6, QG(3, 1), 2, per=1)

                def y_unit(tt, fb, mk=None, act_copy=False):
                    # output projection for one [128 t, 512 f] block: one
                    # accumulation group of 2 full-128-contraction matmuls
                    # (head pair via outT/wo_sb layout), copy to SBUF, DMA
                    f0 = fb * 512
                    if mk is None:
                        yps = ps_y.tile([P, 512], F32, tag="y", name="y_ps")
                    else:
                        yps = mk()
                    for m2 in range(2):
                        nc.tensor.matmul(
                            yps,
                            lhsT=outT[:, m2, tt * P:(tt + 1) * P],
                            rhs=wo_sb[:, m2, f0:f0 + 512],
                            start=(m2 == 0),
                            stop=(m2 == 1),
                            skip_group_check=True,
                        )
                    ysb = p2.tile([P, 512], BF16, tag="ysb", bufs=8, name="ysb")
                    if act_copy:
                        nc.scalar.copy(out=ysb, in_=yps)
                    else:
                        nc.vector.tensor_copy(out=ysb, in_=yps)
                    nc.sync.dma_start(
                        out=y_d[tt * P:(tt + 1) * P, f0:f0 + 512], in_=ysb
                    )

                def emit_scores(m, t0, st):
                    sc = ps_sc.tile([P, 2, TB], F32, tag="sc", name="sc_ps")
                    for hh in range(2):
                        p0 = DK * hh
                        nc.tensor.matmul(
                            sc[:, hh, :],
                            lhsT=KT[p0:p0 + DK, m, st * P:(st + 1) * P],
                            rhs=QT[p0:p0 + DK, m, t0:t0 + TB],
                            start=True,
                            stop=True,
                        )
                    return sc

                halves = [(tb, m) for tb in range(NTB) for m in range(2)]
                pending = []  # deferred y-units of the previous t-block
                sc_cur = emit_scores(halves[0][1], halves[0][0] * TB, 0)
                for hi, (tb, m) in enumerate(halves):
                    t0 = tb * TB
                    pvA = ps_pv.tile([P, TB], F32, tag="pv", name="pvA")
                    pvB = ps_pv.tile([P, TB], F32, tag="pv", name="pvB")

                    def pv_dn(st):
                        for hh, pv in ((0, pvA), (1, pvB)):
                            nc.tensor.matmul(
                                pv[0:DK + 1, :],
                                lhsT=V[:, st, 2 * m + hh, :],
                                rhs=pT[:, st, hh, :],
                                start=(st == 0),
                                stop=(st == NT - 1),
                                skip_group_check=True,
                            )

                    for st in range(NT):
                        # scores are emitted one iteration AHEAD (incl.
                        # across half boundaries) so the exp stream never
                        # waits on PE
                        if st + 1 < NT:
                            sc_nxt = emit_scores(m, t0, st + 1)
                        elif hi + 1 < len(halves):
                            tb2, m2 = halves[hi + 1]
                            sc_nxt = emit_scores(m2, tb2 * TB, 0)
                        else:
                            sc_nxt = None
                        if hi == 0:
                            # V projection from the resident bf16 x, one
                            # s-chunk per iteration, psum via the y bank
                            vps = ps_y.tile([P, 512], F32, tag="y", name="v_ps")
                            for dt in range(8):
                                nc.tensor.matmul(
                                    vps[:, :E],
                                    lhsT=xT_sb[:, dt, st * P:(st + 1) * P],
                                    rhs=wv_sb[:, dt, :],
                                    start=(dt == 0),
                                    stop=(dt == 7),
                                )
                            nc.vector.tensor_copy(
                                out=V[:, st, :, 0:DK], in_=vps[:, :E]
                            )
                        # software pipeline (depth 2): P@V trails exp by two
                        # s-tiles so a new half's first pv matmul never waits
                        # on the previous half's pv banks still draining
                        if st > 1:
                            pv_dn(st - 2)
                        # dripped projection slice for this iteration
                        if hi in drip_plan and st in drip_plan[hi]:
                            g, dts, last = drip_plan[hi][st]
                            g.emit(dts, last)
                        nc.scalar.activation(
                            out=pT[:, st, :, :],
                            in_=sc_cur,
                            func=mybir.ActivationFunctionType.Exp,
                            scale=0.125,
                        )
                        sc_cur = sc_nxt
                        # drip the previous t-block's output projection into
                        # this (ACT-bound) s-loop
                        if pending and st in (5, 8, 11, 14):
                            y_unit(*pending.pop(0))
                    pv_dn(NT - 2)
                    pv_dn(NT - 1)
                    # tail: stage both pv banks to SBUF immediately (frees
                    # the 2-deep psum ring for the next half), 1/den rows ->
                    # DRAM-bounce broadcast, normalize from the SBUF copies;
                    # head-odd is DMA-shifted (sbuf->sbuf) to lanes 64:128
                    ouA = p2.tile([P, TB], F32, tag="ouA", bufs=2, name="ouA")
                    ouB = p2.tile([P, TB], F32R, tag="ouB", bufs=2, name="ouB")
                    den_inv = p2.tile([P, 2, TB], F32, tag="den_inv",
                                      bufs=2, name="den_inv")
                    last = hi == len(halves) - 1
                    if last:
                        # no next half to feed: reciprocals straight from
                        # PSUM, ahead of the staging copies, shorten the
                        # tail's den -> rep -> normalize chain
                        for hh, pv in ((0, pvA), (1, pvB)):
                            nc.vector.reciprocal(
                                out=den_inv[DK:DK + 1, hh, :],
                                in_=pv[DK:DK + 1, :],
                            )
                            nc.sync.dma_start(
                                out=den_dram[2 * tb + m, hh:hh + 1, :],
                                in_=den_inv[DK:DK + 1, hh, :],
                            )
                    nc.vector.tensor_copy(out=ouA[0:DK + 1, :], in_=pvA[0:DK + 1, :])
                    nc.vector.tensor_copy(out=ouB[0:DK + 1, :], in_=pvB[0:DK + 1, :])
                    if not last:
                        # mid-halves read the SBUF staging copy so the pv
                        # bank is released by the copy alone (2-deep ring)
                        for hh, ou in ((0, ouA), (1, ouB)):
                            nc.vector.reciprocal(
                                out=den_inv[DK:DK + 1, hh, :],
                                in_=ou[DK:DK + 1, :],
                            )
                            nc.sync.dma_start(
                                out=den_dram[2 * tb + m, hh:hh + 1, :],
                                in_=den_inv[DK:DK + 1, hh, :],
                            )
                    rep = p2.tile([P, TB], F32, tag="rep", bufs=2, name="rep")
                    for hh in range(2):
                        nc.sync.dma_start(
                            out=rep[DK * hh:DK * hh + DK, :],
                            in_=den_dram[2 * tb + m, hh:hh + 1, :].to_broadcast([DK, TB]),
                        )
                    ou2 = p2.tile([P, TB], F32R, tag="ou2", bufs=2, name="ou2")
                    nc.sync.dma_start(out=ou2[DK:P, :], in_=ouB[0:DK, :])
                    nc.vector.tensor_mul(
                        outT[0:DK, m, t0:t0 + TB], ouA[0:DK, :], rep[0:DK, :]
                    )
                    nc.vector.tensor_mul(
                        outT[DK:P, m, t0:t0 + TB], ou2[DK:P, :], rep[DK:P, :]
                    )
                    if m == 1:
                        pending = [(tt, fb)
                                   for tt in range(tb * (TB // P), (tb + 1) * (TB // P))
                                   for fb in range(2)]
                # tail units: the pv ring, kq and score banks are free now --
                # spread across 6 banks so the units pipeline instead of
                # serializing; ACT is idle after the final exp, so it does
                # the PSUM->SBUF copies
                banks = [
                    lambda: ps_y.tile([P, 512], F32, tag="y", name="y_ps"),
                    lambda: ps_pv.tile([P, TB], F32, tag="pv", name="y_ps"),
                    lambda: ps_pv.tile([P, TB], F32, tag="pv", name="y_ps"),
                    lambda: ps_pv.tile([P, 512], F32, tag="kq", bufs=1, name="y_ps"),
                    lambda: ps_sc.tile([P, 2, TB], F32, tag="sc", name="y_ps")[:, 0, :],
                    lambda: ps_sc.tile([P, 2, TB], F32, tag="sc", name="y_ps")[:, 0, :],
                ]
                # two-pass: the m2=0 matmuls of the first 6 units only need
                # outT[:, 0] (ready since the second-to-last half) -- they
                # run while the last normalize chain drains, keeping the PE
                # p-state warm; m2=1 + copy + DMA complete them after
                staged = []
                for i, (tt, fb) in enumerate(pending[:5]):
                    yps = banks[i]()
                    nc.tensor.matmul(
                        yps,
                        lhsT=outT[:, 0, tt * P:(tt + 1) * P],
                        rhs=wo_sb[:, 0, fb * 512:fb * 512 + 512],
                        start=True,
                        stop=False,
                        skip_group_check=True,
                    )
                    staged.append((yps, tt, fb))
                # keep-warm: the PE would idle ~4us here waiting on the last
                # normalize chain, dropping to the slow p-state right before
                # the final output-projection matmuls; dependency-free dummy
                # matmuls into the remaining score bank keep it at full rate
                warm_ps = banks[5]()
                for i in range(18):
                    nc.tensor.matmul(
                        warm_ps,
                        lhsT=outT[:, 0, 0:P],
                        rhs=wo_sb[:, 0, 0:512],
                        start=True,
                        stop=True,
                        skip_group_check=True,
                    )
                for yps, tt, fb in staged:
                    f0 = fb * 512
                    nc.tensor.matmul(
                        yps,
                        lhsT=outT[:, 1, tt * P:(tt + 1) * P],
                        rhs=wo_sb[:, 1, f0:f0 + 512],
                        start=False,
                        stop=True,
                        skip_group_check=True,
                    )
                    ysb = p2.tile([P, 512], BF16, tag="ysb", bufs=8, name="ysb")
                    nc.scalar.copy(out=ysb, in_=yps)
                    nc.sync.dma_start(
                        out=y_d[tt * P:(tt + 1) * P, f0:f0 + 512], in_=ysb
                    )
                for i, u in enumerate(pending[5:]):
                    y_unit(*u, mk=banks[i % 6], act_copy=(i % 2 == 0))
    _split_multi_waits(nc)
    return nc


def _shard_inputs(x, w_q, b_q, w_k, b_k, w_v, b_v, w_o, b_o):
    import ml_dtypes
    bf16 = ml_dtypes.bfloat16
    in_maps = []
    for c in range(N_CORES):
        b, g = c // 4, c % 4
        sl = slice(g * E, (g + 1) * E)
        in_maps.append({
            "xT": np.ascontiguousarray(x[b].T).astype(bf16),
            "wqT": np.ascontiguousarray(w_q[sl, :].T).astype(bf16),
            "wkT": np.ascontiguousarray(w_k[sl, :].T).astype(bf16),
            "wvT": np.ascontiguousarray(w_v[sl, :].T).astype(bf16),
            "wo_sh": np.ascontiguousarray(w_o[:, sl].T, dtype=np.float32),
            "bq2": np.ascontiguousarray(b_q[sl].reshape(2, P).T, dtype=np.float32),
            "bk2": np.ascontiguousarray(b_k[sl].reshape(2, P).T, dtype=np.float32),
        })
    return in_maps


_NC_CACHE = {}


def kernel(x, w_q, b_q, w_k, b_k, w_v, b_v, w_o, b_o, _trace=False):
    x = np.asarray(x, dtype=np.float32)
    B, T, _ = x.shape
    args = [np.asarray(a, dtype=np.float32)
            for a in (w_q, b_q, w_k, b_k, w_v, b_v, w_o, b_o)]
    w_q, b_q, w_k, b_k, w_v, b_v, w_o, b_o = args

    if T not in _NC_CACHE:
        _NC_CACHE[T] = build_nc(T=T)
    nc = _NC_CACHE[T]
    in_maps = _shard_inputs(x, w_q, b_q, w_k, b_k, w_v, b_v, w_o, b_o)
    res = run_bass_kernel_spmd(nc, in_maps, list(range(N_CORES)), trace=_trace)

    y = np.zeros((B, T, D), dtype=np.float32)
    for c in range(N_CORES):
        y[c // 4] += np.asarray(res.results[c]["y"], dtype=np.float32)
    fold = b_v @ w_o.T + b_o
    y += fold[None, None, :]
    if _trace:
        return y, res
    return y
